# revision 26
# baseline (speedup 1.0000x reference)
"""Trainium2 Bass kernel for HeatmapMaxDetBlock (argmax + local refinement).

Computes, for x[B, C, H, W]:
    scores = max over (H*W); idx = argmax; px = idx % W, py = idx // W (masked
    by score > 0); quarter-pixel refinement by sign of neighbor differences.
Returns [B, C, 3] = (px, py, scores).

Strategy (pure data parallel over 8 NeuronCores, batch-sharded):
  phase 1: stream the whole shard through SBUF once; one DVE reduce_max per
           [128, MD*SEGW] DMA tile produces the per-(row, segment) maxima,
           staying just under the DMA stream rate.
  phase 2: per row group -- PE-transpose the per-(row, segment) maxima, find
           the winning segment (equality trick), one indirect-DMA window
           gather per row, max_index for the in-segment position, then an
           SBUF indirect_copy to pull the 4 refinement neighbors (no second
           DRAM gather).  Row group B (8 rows) is streamed FIRST so its
           entire phase 2 hides under the phase-1 stream; only group A's
           (128 rows) phase-2 chain is exposed at the end.
"""

import sys
from contextlib import ExitStack
from dataclasses import dataclass

import numpy as np

for _p in ("/opt/trn_rl_repo",):
    if _p not in sys.path:
        sys.path.insert(0, _p)

import concourse.bass as bass  # noqa: E402
import concourse.tile as tile  # noqa: E402
from concourse import bacc, bass_isa, mybir  # noqa: E402
from concourse.masks import make_identity  # noqa: E402

F32 = mybir.dt.float32
U16 = mybir.dt.uint16
U32 = mybir.dt.uint32
AX = mybir.AxisListType
OP = mybir.AluOpType

NEG = -3.0e38


@dataclass(frozen=True)
class Cfg:
    B: int = 64
    C: int = 17
    H: int = 256
    W: int = 192
    ncores: int = 8
    P: int = 128
    NSEG: int = 64
    MD: int = 2  # tile-columns merged per DMA
    FRONT: int = 256
    REAR: int = 512

    @property
    def BP(self):  # batches per core
        return self.B // self.ncores

    @property
    def R(self):  # heatmap rows per core
        return self.BP * self.C

    @property
    def HWm(self):
        return self.H * self.W

    @property
    def SEGW(self):
        return self.HWm // self.NSEG

    @property
    def RPT(self):  # rows per tile-column
        return self.P // self.NSEG

    @property
    def NT(self):  # tile-columns per core
        return self.R // self.RPT

    @property
    def MARG(self):
        return self.W + 2

    @property
    def WINW(self):
        return self.SEGW + 2 * self.MARG

    @property
    def SHN(self):
        return self.R * self.HWm

    @property
    def NPAD(self):
        return self.FRONT + self.SHN + self.REAR


CFG = Cfg()


def build_program(cfg: Cfg):
    c = cfg
    assert c.P % c.NSEG == 0 and c.R % c.RPT == 0 and c.HWm % c.NSEG == 0
    assert c.SEGW == 4 * c.W, "px/py decomposition relies on SEGW == 4W"
    assert c.FRONT >= c.MARG and c.REAR >= c.MARG
    assert c.NT % c.MD == 0
    GA = min(c.P, c.R)  # 128 rows in group A
    nta = GA // c.RPT  # 64 tile-columns in group A
    ntb = c.NT - nta  # 4 tile-columns in group B
    HALF = c.SEGW // 2

    nc = bacc.Bacc(
        "TRN2", target_bir_lowering=False, debug=False, num_devices=c.ncores
    )
    xh = nc.dram_tensor("x", [c.NPAD], F32, kind="ExternalInput").ap()
    rbh = nc.dram_tensor("rowbase", [c.NT, c.RPT], F32, kind="ExternalInput").ap()
    irh = nc.dram_tensor("iotarev", [c.NT, c.P], F32, kind="ExternalInput").ap()
    oh = nc.dram_tensor("out", [c.R, 3], F32, kind="ExternalOutput").ap()

    with ExitStack() as ctx:
        tc = ctx.enter_context(tile.TileContext(nc))
        xpool = ctx.enter_context(tc.tile_pool(name="xp", bufs=4))
        sp = ctx.enter_context(tc.tile_pool(name="sp", bufs=1))
        pp = ctx.enter_context(tc.tile_pool(name="pp", bufs=1, space="PSUM"))

        # ---- long-lived tiles -----------------------------------------------
        M = sp.tile([c.P, c.NT], F32, tag="M")  # per-(row,seg) maxima
        ident = sp.tile([c.P, c.P], F32, tag="ident")
        irt = sp.tile([c.NT, c.P], F32, tag="irt")
        rbt = sp.tile([c.NT, c.RPT], F32, tag="rbt")
        rbtB = sp.tile([ntb, c.RPT], F32, tag="rbtB")
        c3 = sp.tile([c.P, 3], F32, tag="c3")  # {192,384,576}
        hi2 = sp.tile([c.P, 2], F32, tag="hi2")  # {W-1, H-1}

        # ---- startup constants (all off the DMA critical path) --------------
        make_identity(nc, ident[:])
        nc.sync.dma_start(out=irt[:], in_=irh[:])
        nc.sync.dma_start(out=rbt[:], in_=rbh[:])
        nc.sync.dma_start(out=rbtB[:], in_=rbh[nta : c.NT])
        for i, v in enumerate((192.0, 384.0, 576.0)):
            nc.gpsimd.memset(c3[:, i : i + 1], v)
        nc.gpsimd.memset(hi2[:, 0:1], float(c.W - 1))
        nc.gpsimd.memset(hi2[:, 1:2], float(c.H - 1))

        # ---- phase 1 helpers -------------------------------------------------
        def emit_dma(t0, eng):
            xt = xpool.tile([c.P, c.MD * c.SEGW], F32, tag="xt")
            off = c.FRONT + t0 * c.RPT * c.HWm
            src = bass.AP(
                xh.tensor,
                off,
                [
                    [c.HWm, c.RPT],
                    [c.SEGW, c.NSEG],
                    [c.RPT * c.HWm, c.MD],
                    [1, c.SEGW],
                ],
            )
            eng.dma_start(
                out=xt[:].rearrange("p (m u) -> p m u", m=c.MD), in_=src
            )
            return xt[:].rearrange("p (m u) -> p m u", m=c.MD)

        def emit_ttr(xt3, t0):
            nc.vector.reduce_max(
                out=M[:, t0 : t0 + c.MD], in_=xt3, axis=AX.X
            )

        # ---- phase 2 pieces --------------------------------------------------
        def seg_find(eng, MT_ap, t_lo, t_hi, P43, sb, rbt_ap):
            """Winning segment + scores for tile-columns [t_lo, t_hi).
            MT_ap: [t_hi-t_lo, P] transposed maxima (partition = tile-column).
            rbt_ap: [t_hi-t_lo, RPT] rowbase slice based at partition 0.
            Writes P43 [(t), RPT, 3] = (w0, score, 4*seg)."""
            n = t_hi - t_lo
            MT3 = MT_ap.rearrange("p (j s) -> p j s", j=c.RPT)
            sc = P43[:, :, 1:2]
            eng.tensor_reduce(out=sc, in_=MT3, axis=AX.X, op=OP.max)
            mk = sp.tile([n, c.P], F32, tag=f"mk{t_lo}")
            mk3 = mk[:].rearrange("p (j s) -> p j s", j=c.RPT)
            eng.tensor_tensor(
                out=mk3,
                in0=MT3,
                in1=sc.to_broadcast([n, c.RPT, c.NSEG]),
                op=OP.is_equal,
            )
            # iotarev rows are identical, so the partition-0-based slice works
            # for any tile-column range.
            eng.tensor_tensor(
                out=mk3,
                in0=mk3,
                in1=irt[0:n].rearrange("p (j s) -> p j s", j=c.RPT),
                op=OP.mult,
            )
            eng.tensor_reduce(out=sb[:], in_=mk3, axis=AX.X, op=OP.max)
            # sb := seg_base = (NSEG-1 - srev) * SEGW
            eng.tensor_scalar(
                out=sb[:],
                in0=sb[:],
                scalar1=-float(c.SEGW),
                scalar2=float((c.NSEG - 1) * c.SEGW),
                op0=OP.mult,
                op1=OP.add,
            )
            eng.tensor_tensor(
                out=P43[:, :, 0:1],
                in0=sb[:, :, None],
                in1=rbt_ap[:, :, None],
                op=OP.add,
            )
            # 4*seg = seg_base / W  (exact in f32)
            eng.tensor_scalar(
                out=P43[:, :, 2:3],
                in0=sb[:, :, None],
                scalar1=1.0 / c.W,
                scalar2=None,
                op0=OP.mult,
            )

        def row_chain_pre(eng, Rt, gp, tag):
            """w0 -> window gather (gp rows on partitions)."""
            w0u = sp.tile([gp, 1], U32, tag=f"w0u{tag}")
            eng.tensor_copy(out=w0u[:], in_=Rt[:, 0:1])
            win = sp.tile([gp, c.WINW], F32, tag=f"win{tag}")
            nc.gpsimd.indirect_dma_start(
                out=win[:],
                out_offset=None,
                in_=xh[:, None],
                in_offset=bass.IndirectOffsetOnAxis(ap=w0u[:, 0:1], axis=0),
            )
            m8 = sp.tile([gp, 8], F32, tag=f"m8{tag}")
            eng.tensor_copy(out=m8[:], in_=Rt[:, 1:2].to_broadcast([gp, 8]))
            return win, m8

        def row_chain_find(Rt, gp, tag, win, m8, clamp):
            """max_index (vector only) -> ii (f32)."""
            mi = sp.tile([gp, 8], U32, tag=f"mi{tag}")
            nc.vector.max_index(
                mi[:], m8[:], win[:, c.MARG : c.MARG + c.SEGW]
            )
            ii = sp.tile([gp, 1], F32, tag=f"ii{tag}")
            nc.vector.tensor_copy(out=ii[:], in_=mi[:, 0:1])
            if clamp:
                nc.vector.tensor_scalar(
                    out=ii[:], in0=ii[:], scalar1=float(c.SEGW - 1),
                    scalar2=None, op0=OP.min,
                )
            return ii

        NBW = 2 * c.W + 1

        def row_chain_nb(eng, Rt, gp, tag, ii):
            """DRAM gather of the (peak-W .. peak+W) neighborhood.
            nb[k] = x[peak - W + k]; nb start = w0 + ii + 2."""
            w2 = sp.tile([gp, 1], F32, tag=f"w2{tag}")
            eng.tensor_tensor(out=w2[:], in0=Rt[:, 0:1], in1=ii[:], op=OP.add)
            w2u = sp.tile([gp, 1], U32, tag=f"w2u{tag}")
            eng.tensor_scalar(
                out=w2u[:], in0=w2[:], scalar1=2.0, scalar2=None, op0=OP.add
            )
            nb = sp.tile([gp, NBW], F32, tag=f"nb{tag}")
            nc.gpsimd.indirect_dma_start(
                out=nb[:],
                out_offset=None,
                in_=xh[:, None],
                in_offset=bass.IndirectOffsetOnAxis(ap=w2u[:, 0:1], axis=0),
            )
            return nb

        def row_chain_math(eng, Rt, gp, tag, ii, O):
            """px/py/mask/interior (no nb4 dependency)."""
            cg = sp.tile([gp, 3], F32, tag=f"cg{tag}")
            eng.tensor_tensor(
                out=cg[:], in0=ii[:].to_broadcast([gp, 3]), in1=c3[0:gp],
                op=OP.is_ge,
            )
            qf = sp.tile([gp, 1], F32, tag=f"qf{tag}")
            eng.tensor_reduce(out=qf[:], in_=cg[:], axis=AX.X, op=OP.add)
            # py = 4*seg + floor(ii/W);  px = ii - W*floor(ii/W)
            eng.tensor_tensor(out=O[:, 1:2], in0=Rt[:, 2:3], in1=qf[:], op=OP.add)
            t2 = sp.tile([gp, 1], F32, tag=f"t2{tag}")
            eng.tensor_scalar(
                out=t2[:], in0=qf[:], scalar1=-float(c.W), scalar2=None,
                op0=OP.mult,
            )
            eng.tensor_tensor(out=O[:, 0:1], in0=ii[:], in1=t2[:], op=OP.add)
            mk1 = sp.tile([gp, 1], F32, tag=f"mk1{tag}")
            eng.tensor_scalar(
                out=mk1[:], in0=Rt[:, 1:2], scalar1=0.0, scalar2=None,
                op0=OP.is_gt,
            )
            eng.tensor_tensor(
                out=O[:, 0:2], in0=O[:, 0:2],
                in1=mk1[:].to_broadcast([gp, 2]), op=OP.mult,
            )
            ig = sp.tile([gp, 2], F32, tag=f"ig{tag}")
            eng.tensor_scalar(
                out=ig[:], in0=O[:, 0:2], scalar1=0.0, scalar2=None,
                op0=OP.is_gt,
            )
            il = sp.tile([gp, 2], F32, tag=f"il{tag}")
            eng.tensor_tensor(out=il[:], in0=O[:, 0:2], in1=hi2[0:gp], op=OP.is_lt)
            eng.tensor_tensor(out=ig[:], in0=ig[:], in1=il[:], op=OP.mult)
            intr = sp.tile([gp, 1], F32, tag=f"intr{tag}")
            eng.tensor_reduce(out=intr[:], in_=ig[:], axis=AX.X, op=OP.min)
            eng.tensor_copy(out=O[:, 2:3], in_=Rt[:, 1:2])
            return intr

        def row_chain_refine(eng, gp, tag, nb, intr, O):
            """dx/dy from neighbors, apply.
            nb[W-1]=x[c-1], nb[W+1]=x[c+1], nb[0]=x[c-W], nb[2W]=x[c+W]."""
            dg = sp.tile([gp, 2], F32, tag=f"dg{tag}")
            dl = sp.tile([gp, 2], F32, tag=f"dl{tag}")
            for a, (ir, il) in enumerate(((c.W + 1, c.W - 1), (2 * c.W, 0))):
                eng.tensor_tensor(
                    out=dg[:, a : a + 1], in0=nb[:, ir : ir + 1],
                    in1=nb[:, il : il + 1], op=OP.is_gt,
                )
                eng.tensor_tensor(
                    out=dl[:, a : a + 1], in0=nb[:, ir : ir + 1],
                    in1=nb[:, il : il + 1], op=OP.is_lt,
                )
            eng.tensor_tensor(out=dg[:], in0=dg[:], in1=dl[:], op=OP.subtract)
            eng.tensor_scalar(
                out=dg[:], in0=dg[:], scalar1=0.25, scalar2=None, op0=OP.mult
            )
            eng.tensor_tensor(
                out=dg[:], in0=dg[:], in1=intr[:].to_broadcast([gp, 2]),
                op=OP.mult,
            )
            eng.tensor_tensor(out=O[:, 0:2], in0=O[:, 0:2], in1=dg[:], op=OP.add)

        # ---------------------------------------------------------------------
        # B-group tiles (rows 128..135, tile-columns 64..68) stream FIRST.
        # ---------------------------------------------------------------------
        RB = sp.tile([16, 3], F32, tag="RB")
        nc.gpsimd.memset(RB[:], 0.0)
        OB = sp.tile([16, 3], F32, tag="OB")

        dma_order = [nta + 2 * i for i in range(ntb // c.MD)] + [
            2 * i for i in range(nta // c.MD)
        ]
        engs = [nc.sync, nc.scalar]

        # B stream + reduce
        for i, t0 in enumerate(dma_order[: ntb // c.MD]):
            emit_ttr(emit_dma(t0, engs[i % 2]), t0)

        # B transpose (PE) + MTB copy (vector)
        ppB = pp.tile([ntb, c.P], F32)
        nc.tensor.transpose(out=ppB[:], in_=M[:, nta : c.NT], identity=ident[:])
        MTB = sp.tile([ntb, c.P], F32, tag="MTB")
        nc.vector.tensor_copy(out=MTB[:], in_=ppB[:])

        # A stream, with B's phase-2 chain interleaved into vector slack
        P43B = sp.tile([ntb, c.RPT * 3], F32, tag="P43B")
        P43B3 = P43B[:].rearrange("p (j e) -> p j e", e=3)
        sbB = sp.tile([ntb, c.RPT], F32, tag="sbB")
        winB = m8B = iiB = nbB = intrB = None
        na = nta // c.MD
        for i, t0 in enumerate(dma_order[ntb // c.MD :]):
            emit_ttr(emit_dma(t0, engs[i % 2]), t0)
            if i == 0:
                seg_find(nc.vector, MTB[:], nta, c.NT, P43B3, sbB, rbtB[:])
                nc.gpsimd.dma_start(out=RB[0 : 2 * ntb], in_=P43B3)
            elif i == 1:
                winB, m8B = row_chain_pre(nc.vector, RB[:], 16, "b")
            elif i == 3:
                iiB = row_chain_find(RB[:], 16, "b", winB, m8B, clamp=True)
                nbB = row_chain_nb(nc.vector, RB[:], 16, "b", iiB)
            elif i == 5:
                intrB = row_chain_math(nc.vector, RB[:], 16, "b", iiB, OB[:])
                row_chain_refine(nc.vector, 16, "b", nbB, intrB, OB[:])
            elif i == 7:
                nc.gpsimd.dma_start(out=oh[GA : c.R], in_=OB[0 : c.R - GA])

        # ---------------------------------------------------------------------
        # A group (rows 0..127) -- the exposed tail.
        # ---------------------------------------------------------------------
        ppA = pp.tile([nta, c.P], F32)
        nc.tensor.transpose(out=ppA[:], in_=M[:, 0:nta], identity=ident[:])
        MTA = sp.tile([nta, c.P], F32, tag="MTA")
        nc.vector.tensor_copy(out=MTA[:], in_=ppA[:])

        P43A = sp.tile([nta, c.RPT * 3], F32, tag="P43A")
        P43A3 = P43A[:].rearrange("p (j e) -> p j e", e=3)
        sbA = sp.tile([nta, c.RPT], F32, tag="sbA")
        seg_find(nc.vector, MTA[:], 0, nta, P43A3, sbA, rbt[0:nta])
        RA = sp.tile([GA, 3], F32, tag="RA")
        nc.scalar.dma_start(out=RA[:], in_=P43A3)

        winA, m8A = row_chain_pre(nc.vector, RA[:], GA, "a")
        iiA = row_chain_find(RA[:], GA, "a", winA, m8A, clamp=False)
        nbA = row_chain_nb(nc.vector, RA[:], GA, "a", iiA)
        OA = sp.tile([GA, 3], F32, tag="OA")
        intrA = row_chain_math(nc.vector, RA[:], GA, "a", iiA, OA[:])
        row_chain_refine(nc.vector, GA, "a", nbA, intrA, OA[:])
        nc.scalar.dma_start(out=oh[0:GA], in_=OA[:])

    nc.compile()
    return nc


def host_constants(cfg: Cfg):
    c = cfg
    r = np.arange(c.R, dtype=np.float64)
    rowbase = (c.FRONT + r * c.HWm - c.MARG).astype(np.float32).reshape(c.NT, c.RPT)
    s = np.arange(c.NSEG, dtype=np.float64)
    row = np.tile((c.NSEG - 1 - s), c.RPT).astype(np.float32)  # [P]
    iotarev = np.tile(row, (c.NT, 1)).astype(np.float32)
    return rowbase, iotarev


def shard_inputs(cfg: Cfg, x: np.ndarray):
    c = cfg
    rowbase, iotarev = host_constants(c)
    in_maps = []
    for k in range(c.ncores):
        shard = np.ascontiguousarray(
            x[k * c.BP : (k + 1) * c.BP], dtype=np.float32
        ).reshape(-1)
        xp = np.zeros(c.NPAD, np.float32)
        xp[c.FRONT : c.FRONT + c.SHN] = shard
        in_maps.append({"x": xp, "rowbase": rowbase, "iotarev": iotarev})
    return in_maps


def assemble_out(cfg: Cfg, per_core_outs):
    c = cfg
    outs = [o.reshape(c.BP, c.C, 3).astype(np.float32) for o in per_core_outs]
    return np.concatenate(outs, axis=0)


_PROGRAM = None


def _program():
    global _PROGRAM
    if _PROGRAM is None:
        _PROGRAM = build_program(CFG)
    return _PROGRAM


def kernel(x: np.ndarray) -> np.ndarray:
    from concourse.bass_utils import run_bass_kernel_spmd

    c = CFG
    assert x.shape == (c.B, c.C, c.H, c.W), x.shape
    nc = _program()
    in_maps = shard_inputs(c, np.asarray(x))
    res = run_bass_kernel_spmd(nc, in_maps, core_ids=list(range(c.ncores)))
    return assemble_out(c, [res.results[k]["out"] for k in range(c.ncores)])


# revision 27
# speedup vs baseline: 1.0320x; 1.0320x over previous
"""Trainium2 Bass kernel for HeatmapMaxDetBlock (argmax + local refinement).

Computes, for x[B, C, H, W]:
    scores = max over (H*W); idx = argmax; px = idx % W, py = idx // W (masked
    by score > 0); quarter-pixel refinement by sign of neighbor differences.
Returns [B, C, 3] = (px, py, scores).

Strategy (pure data parallel over 8 NeuronCores, batch-sharded):
  phase 1: stream the whole shard through SBUF once; one DVE reduce_max per
           [128, MD*SEGW] DMA tile produces the per-(row, segment) maxima,
           staying just under the DMA stream rate.
  phase 2: per row group -- PE-transpose the per-(row, segment) maxima, find
           the winning segment (equality trick), one indirect-DMA window
           gather per row, max_index for the in-segment position, then an
           SBUF indirect_copy to pull the 4 refinement neighbors (no second
           DRAM gather).  Row group B (8 rows) is streamed FIRST so its
           entire phase 2 hides under the phase-1 stream; only group A's
           (128 rows) phase-2 chain is exposed at the end.
"""

import sys
from contextlib import ExitStack
from dataclasses import dataclass

import numpy as np

for _p in ("/opt/trn_rl_repo",):
    if _p not in sys.path:
        sys.path.insert(0, _p)

import concourse.bass as bass  # noqa: E402
import concourse.tile as tile  # noqa: E402
from concourse import bacc, bass_isa, mybir  # noqa: E402
from concourse.masks import make_identity  # noqa: E402

F32 = mybir.dt.float32
U16 = mybir.dt.uint16
U32 = mybir.dt.uint32
AX = mybir.AxisListType
OP = mybir.AluOpType

NEG = -3.0e38


@dataclass(frozen=True)
class Cfg:
    B: int = 64
    C: int = 17
    H: int = 256
    W: int = 192
    ncores: int = 8
    P: int = 128
    NSEG: int = 64
    MD: int = 4  # tile-columns merged per DMA
    FRONT: int = 256
    REAR: int = 512

    @property
    def BP(self):  # batches per core
        return self.B // self.ncores

    @property
    def R(self):  # heatmap rows per core
        return self.BP * self.C

    @property
    def HWm(self):
        return self.H * self.W

    @property
    def SEGW(self):
        return self.HWm // self.NSEG

    @property
    def RPT(self):  # rows per tile-column
        return self.P // self.NSEG

    @property
    def NT(self):  # tile-columns per core
        return self.R // self.RPT

    @property
    def MARG(self):
        return self.W + 2

    @property
    def WINW(self):
        return self.SEGW

    @property
    def SHN(self):
        return self.R * self.HWm

    @property
    def NPAD(self):
        return self.FRONT + self.SHN + self.REAR


CFG = Cfg()


def build_program(cfg: Cfg):
    c = cfg
    assert c.P % c.NSEG == 0 and c.R % c.RPT == 0 and c.HWm % c.NSEG == 0
    assert c.SEGW == 4 * c.W, "px/py decomposition relies on SEGW == 4W"
    assert c.FRONT >= c.MARG and c.REAR >= c.MARG
    assert c.NT % c.MD == 0
    GA = min(c.P, c.R)  # 128 rows in group A
    nta = GA // c.RPT  # 64 tile-columns in group A
    ntb = c.NT - nta  # 4 tile-columns in group B
    HALF = c.SEGW // 2

    nc = bacc.Bacc(
        "TRN2", target_bir_lowering=False, debug=False, num_devices=c.ncores
    )
    xh = nc.dram_tensor("x", [c.NPAD], F32, kind="ExternalInput").ap()
    rbh = nc.dram_tensor("rowbase", [c.NT, c.RPT], F32, kind="ExternalInput").ap()
    irh = nc.dram_tensor("iotarev", [c.NT, c.P], F32, kind="ExternalInput").ap()
    oh = nc.dram_tensor("out", [c.R, 3], F32, kind="ExternalOutput").ap()

    with ExitStack() as ctx:
        tc = ctx.enter_context(tile.TileContext(nc))
        xpool = ctx.enter_context(tc.tile_pool(name="xp", bufs=4))
        sp = ctx.enter_context(tc.tile_pool(name="sp", bufs=1))
        pp = ctx.enter_context(tc.tile_pool(name="pp", bufs=1, space="PSUM"))

        # ---- long-lived tiles -----------------------------------------------
        M = sp.tile([c.P, c.NT], F32, tag="M")  # per-(row,seg) maxima
        ident = sp.tile([c.P, c.P], F32, tag="ident")
        irt = sp.tile([c.NT, c.P], F32, tag="irt")
        rbt = sp.tile([c.NT, c.RPT], F32, tag="rbt")
        rbtB = sp.tile([ntb, c.RPT], F32, tag="rbtB")
        c3 = sp.tile([c.P, 3], F32, tag="c3")  # {192,384,576}
        hi2 = sp.tile([c.P, 2], F32, tag="hi2")  # {W-1, H-1}

        # ---- startup constants (all off the DMA critical path) --------------
        make_identity(nc, ident[:])
        # constants go through the gpsimd SWDGE queue so the sync/scalar
        # HWDGE queues start streaming x immediately
        nc.gpsimd.dma_start(out=irt[:], in_=irh[:])
        nc.gpsimd.dma_start(out=rbt[:], in_=rbh[:])
        nc.gpsimd.dma_start(out=rbtB[:], in_=rbh[nta : c.NT])
        for i, v in enumerate((192.0, 384.0, 576.0)):
            nc.gpsimd.memset(c3[:, i : i + 1], v)
        nc.gpsimd.memset(hi2[:, 0:1], float(c.W - 1))
        nc.gpsimd.memset(hi2[:, 1:2], float(c.H - 1))

        # ---- phase 1 helpers -------------------------------------------------
        def emit_dma(t0, eng):
            xt = xpool.tile([c.P, c.MD * c.SEGW], F32, tag="xt")
            off = c.FRONT + t0 * c.RPT * c.HWm
            src = bass.AP(
                xh.tensor,
                off,
                [
                    [c.HWm, c.RPT],
                    [c.SEGW, c.NSEG],
                    [c.RPT * c.HWm, c.MD],
                    [1, c.SEGW],
                ],
            )
            eng.dma_start(
                out=xt[:].rearrange("p (m u) -> p m u", m=c.MD), in_=src
            )
            return xt[:].rearrange("p (m u) -> p m u", m=c.MD)

        def emit_ttr(xt3, t0):
            nc.vector.reduce_max(
                out=M[:, t0 : t0 + c.MD], in_=xt3, axis=AX.X
            )

        # ---- phase 2 pieces --------------------------------------------------
        def seg_find(eng, MT_ap, t_lo, t_hi, P43, sb, rbt_ap):
            """Winning segment + scores for tile-columns [t_lo, t_hi).
            MT_ap: [t_hi-t_lo, P] transposed maxima (partition = tile-column).
            rbt_ap: [t_hi-t_lo, RPT] rowbase slice based at partition 0.
            Writes P43 [(t), RPT, 3] = (w0, score, 4*seg)."""
            n = t_hi - t_lo
            MT3 = MT_ap.rearrange("p (j s) -> p j s", j=c.RPT)
            sc = P43[:, :, 1:2]
            eng.tensor_reduce(out=sc, in_=MT3, axis=AX.X, op=OP.max)
            mk = sp.tile([n, c.P], F32, tag=f"mk{t_lo}")
            mk3 = mk[:].rearrange("p (j s) -> p j s", j=c.RPT)
            eng.tensor_tensor(
                out=mk3,
                in0=MT3,
                in1=sc.to_broadcast([n, c.RPT, c.NSEG]),
                op=OP.is_equal,
            )
            # iotarev rows are identical, so the partition-0-based slice works
            # for any tile-column range.
            eng.tensor_tensor(
                out=mk3,
                in0=mk3,
                in1=irt[0:n].rearrange("p (j s) -> p j s", j=c.RPT),
                op=OP.mult,
            )
            eng.tensor_reduce(out=sb[:], in_=mk3, axis=AX.X, op=OP.max)
            # sb := seg_base = (NSEG-1 - srev) * SEGW
            eng.tensor_scalar(
                out=sb[:],
                in0=sb[:],
                scalar1=-float(c.SEGW),
                scalar2=float((c.NSEG - 1) * c.SEGW),
                op0=OP.mult,
                op1=OP.add,
            )
            eng.tensor_tensor(
                out=P43[:, :, 0:1],
                in0=sb[:, :, None],
                in1=rbt_ap[:, :, None],
                op=OP.add,
            )
            # 4*seg = seg_base / W  (exact in f32)
            eng.tensor_scalar(
                out=P43[:, :, 2:3],
                in0=sb[:, :, None],
                scalar1=1.0 / c.W,
                scalar2=None,
                op0=OP.mult,
            )

        def row_chain_pre(eng, Rt, gp, tag):
            """w0 -> window gather (gp rows on partitions)."""
            w0u = sp.tile([gp, 1], U32, tag=f"w0u{tag}")
            eng.tensor_copy(out=w0u[:], in_=Rt[:, 0:1])
            win = sp.tile([gp, c.WINW], F32, tag=f"win{tag}")
            nc.gpsimd.indirect_dma_start(
                out=win[:],
                out_offset=None,
                in_=xh[:, None],
                in_offset=bass.IndirectOffsetOnAxis(ap=w0u[:, 0:1], axis=0),
            )
            m8 = sp.tile([gp, 8], F32, tag=f"m8{tag}")
            eng.tensor_copy(out=m8[:], in_=Rt[:, 1:2].to_broadcast([gp, 8]))
            return win, m8

        def row_chain_find(Rt, gp, tag, win, m8, clamp):
            """max_index (vector only) -> ii (f32)."""
            mi = sp.tile([gp, 8], U32, tag=f"mi{tag}")
            nc.vector.max_index(mi[:], m8[:], win[:])
            ii = sp.tile([gp, 1], F32, tag=f"ii{tag}")
            nc.vector.tensor_copy(out=ii[:], in_=mi[:, 0:1])
            if clamp:
                nc.vector.tensor_scalar(
                    out=ii[:], in0=ii[:], scalar1=float(c.SEGW - 1),
                    scalar2=None, op0=OP.min,
                )
            return ii

        NBW = 2 * c.W + 1

        def row_chain_nb(eng, Rt, gp, tag, ii):
            """DRAM gather of the (peak-W .. peak+W) neighborhood.
            nb[k] = x[peak - W + k]; nb start = w0 + ii - W."""
            w2u = sp.tile([gp, 1], U32, tag=f"w2u{tag}")
            eng.scalar_tensor_tensor(
                out=w2u[:], in0=Rt[:, 0:1], scalar=-float(c.W), in1=ii[:],
                op0=OP.add, op1=OP.add,
            )
            nb = sp.tile([gp, NBW], F32, tag=f"nb{tag}")
            nc.gpsimd.indirect_dma_start(
                out=nb[:],
                out_offset=None,
                in_=xh[:, None],
                in_offset=bass.IndirectOffsetOnAxis(ap=w2u[:, 0:1], axis=0),
            )
            return nb

        def row_chain_math(eng, Rt, gp, tag, ii, O):
            """px/py/mask/interior (no nb4 dependency)."""
            cg = sp.tile([gp, 3], F32, tag=f"cg{tag}")
            eng.tensor_tensor(
                out=cg[:], in0=ii[:].to_broadcast([gp, 3]), in1=c3[0:gp],
                op=OP.is_ge,
            )
            qf = sp.tile([gp, 1], F32, tag=f"qf{tag}")
            eng.tensor_reduce(out=qf[:], in_=cg[:], axis=AX.X, op=OP.add)
            # py = 4*seg + floor(ii/W);  px = ii - W*floor(ii/W)
            eng.tensor_tensor(out=O[:, 1:2], in0=Rt[:, 2:3], in1=qf[:], op=OP.add)
            t2 = sp.tile([gp, 1], F32, tag=f"t2{tag}")
            eng.tensor_scalar(
                out=t2[:], in0=qf[:], scalar1=-float(c.W), scalar2=None,
                op0=OP.mult,
            )
            eng.tensor_tensor(out=O[:, 0:1], in0=ii[:], in1=t2[:], op=OP.add)
            mk1 = sp.tile([gp, 1], F32, tag=f"mk1{tag}")
            eng.tensor_scalar(
                out=mk1[:], in0=Rt[:, 1:2], scalar1=0.0, scalar2=None,
                op0=OP.is_gt,
            )
            eng.tensor_tensor(
                out=O[:, 0:2], in0=O[:, 0:2],
                in1=mk1[:].to_broadcast([gp, 2]), op=OP.mult,
            )
            ig = sp.tile([gp, 2], F32, tag=f"ig{tag}")
            eng.tensor_scalar(
                out=ig[:], in0=O[:, 0:2], scalar1=0.0, scalar2=None,
                op0=OP.is_gt,
            )
            il = sp.tile([gp, 2], F32, tag=f"il{tag}")
            eng.tensor_tensor(out=il[:], in0=O[:, 0:2], in1=hi2[0:gp], op=OP.is_lt)
            eng.tensor_tensor(out=ig[:], in0=ig[:], in1=il[:], op=OP.mult)
            intr = sp.tile([gp, 1], F32, tag=f"intr{tag}")
            eng.tensor_reduce(out=intr[:], in_=ig[:], axis=AX.X, op=OP.min)
            eng.tensor_copy(out=O[:, 2:3], in_=Rt[:, 1:2])
            return intr

        def row_chain_refine(eng, gp, tag, nb, intr, O):
            """dx/dy from neighbors, apply.
            nb[W-1]=x[c-1], nb[W+1]=x[c+1], nb[0]=x[c-W], nb[2W]=x[c+W]."""
            dg = sp.tile([gp, 2], F32, tag=f"dg{tag}")
            dl = sp.tile([gp, 2], F32, tag=f"dl{tag}")
            for a, (ir, il) in enumerate(((c.W + 1, c.W - 1), (2 * c.W, 0))):
                eng.tensor_tensor(
                    out=dg[:, a : a + 1], in0=nb[:, ir : ir + 1],
                    in1=nb[:, il : il + 1], op=OP.is_gt,
                )
                eng.tensor_tensor(
                    out=dl[:, a : a + 1], in0=nb[:, ir : ir + 1],
                    in1=nb[:, il : il + 1], op=OP.is_lt,
                )
            eng.tensor_tensor(out=dg[:], in0=dg[:], in1=dl[:], op=OP.subtract)
            eng.tensor_scalar(
                out=dg[:], in0=dg[:], scalar1=0.25, scalar2=None, op0=OP.mult
            )
            eng.tensor_tensor(
                out=dg[:], in0=dg[:], in1=intr[:].to_broadcast([gp, 2]),
                op=OP.mult,
            )
            eng.tensor_tensor(out=O[:, 0:2], in0=O[:, 0:2], in1=dg[:], op=OP.add)

        # ---------------------------------------------------------------------
        # B-group tiles (rows 128..135, tile-columns 64..68) stream FIRST.
        # ---------------------------------------------------------------------
        RB = sp.tile([16, 3], F32, tag="RB")
        nc.gpsimd.memset(RB[:], 192.0)
        OB = sp.tile([16, 3], F32, tag="OB")

        dma_order = [nta + c.MD * i for i in range(ntb // c.MD)] + [
            c.MD * i for i in range(nta // c.MD)
        ]
        # B-chain interleave points (A-DMA loop indices), scaled to DMA size
        ip = {k: max(1, v * 2 // c.MD) if k else 0
              for k, v in (("", 0), ("pre", 2), ("find", 6), ("math", 10),
                           ("out", 14))}
        engs = [nc.sync, nc.scalar]

        # B stream + reduce
        for i, t0 in enumerate(dma_order[: ntb // c.MD]):
            emit_ttr(emit_dma(t0, engs[i % 2]), t0)

        # B transpose (PE) + MTB copy (vector)
        ppB = pp.tile([ntb, c.P], F32)
        nc.tensor.transpose(out=ppB[:], in_=M[:, nta : c.NT], identity=ident[:])
        MTB = sp.tile([ntb, c.P], F32, tag="MTB")
        nc.vector.tensor_copy(out=MTB[:], in_=ppB[:])

        # A stream, with B's phase-2 chain interleaved into vector slack
        P43B = sp.tile([ntb, c.RPT * 3], F32, tag="P43B")
        P43B3 = P43B[:].rearrange("p (j e) -> p j e", e=3)
        sbB = sp.tile([ntb, c.RPT], F32, tag="sbB")
        winB = m8B = iiB = nbB = intrB = None
        na = nta // c.MD
        for i, t0 in enumerate(dma_order[ntb // c.MD :]):
            emit_ttr(emit_dma(t0, engs[i % 2]), t0)
            if i == ip[""]:
                seg_find(nc.vector, MTB[:], nta, c.NT, P43B3, sbB, rbtB[:])
                nc.gpsimd.dma_start(out=RB[0 : 2 * ntb], in_=P43B3)
            elif i == ip["pre"]:
                winB, m8B = row_chain_pre(nc.vector, RB[:], 16, "b")
            elif i == ip["find"]:
                iiB = row_chain_find(RB[:], 16, "b", winB, m8B, clamp=True)
                nbB = row_chain_nb(nc.vector, RB[:], 16, "b", iiB)
            elif i == ip["math"]:
                intrB = row_chain_math(nc.vector, RB[:], 16, "b", iiB, OB[:])
                row_chain_refine(nc.vector, 16, "b", nbB, intrB, OB[:])
            elif i == ip["out"]:
                nc.gpsimd.dma_start(out=oh[GA : c.R], in_=OB[0 : c.R - GA])

        # ---------------------------------------------------------------------
        # A group (rows 0..127) -- the exposed tail.
        # ---------------------------------------------------------------------
        ppA = pp.tile([nta, c.P], F32)
        nc.tensor.transpose(out=ppA[:], in_=M[:, 0:nta], identity=ident[:])
        MTA = sp.tile([nta, c.P], F32, tag="MTA")
        nc.vector.tensor_copy(out=MTA[:], in_=ppA[:])

        P43A = sp.tile([nta, c.RPT * 3], F32, tag="P43A")
        P43A3 = P43A[:].rearrange("p (j e) -> p j e", e=3)
        sbA = sp.tile([nta, c.RPT], F32, tag="sbA")
        seg_find(nc.vector, MTA[:], 0, nta, P43A3, sbA, rbt[0:nta])
        RA = sp.tile([GA, 3], F32, tag="RA")
        nc.scalar.dma_start(out=RA[:], in_=P43A3)

        winA, m8A = row_chain_pre(nc.vector, RA[:], GA, "a")
        iiA = row_chain_find(RA[:], GA, "a", winA, m8A, clamp=False)
        nbA = row_chain_nb(nc.vector, RA[:], GA, "a", iiA)
        OA = sp.tile([GA, 3], F32, tag="OA")
        intrA = row_chain_math(nc.vector, RA[:], GA, "a", iiA, OA[:])
        row_chain_refine(nc.vector, GA, "a", nbA, intrA, OA[:])
        nc.scalar.dma_start(out=oh[0:GA], in_=OA[:])

    nc.compile()
    return nc


def host_constants(cfg: Cfg):
    c = cfg
    r = np.arange(c.R, dtype=np.float64)
    rowbase = (c.FRONT + r * c.HWm).astype(np.float32).reshape(c.NT, c.RPT)
    s = np.arange(c.NSEG, dtype=np.float64)
    row = np.tile((c.NSEG - 1 - s), c.RPT).astype(np.float32)  # [P]
    iotarev = np.tile(row, (c.NT, 1)).astype(np.float32)
    return rowbase, iotarev


def shard_inputs(cfg: Cfg, x: np.ndarray):
    c = cfg
    rowbase, iotarev = host_constants(c)
    in_maps = []
    for k in range(c.ncores):
        shard = np.ascontiguousarray(
            x[k * c.BP : (k + 1) * c.BP], dtype=np.float32
        ).reshape(-1)
        xp = np.zeros(c.NPAD, np.float32)
        xp[c.FRONT : c.FRONT + c.SHN] = shard
        in_maps.append({"x": xp, "rowbase": rowbase, "iotarev": iotarev})
    return in_maps


def assemble_out(cfg: Cfg, per_core_outs):
    c = cfg
    outs = [o.reshape(c.BP, c.C, 3).astype(np.float32) for o in per_core_outs]
    return np.concatenate(outs, axis=0)


_PROGRAM = None


def _program():
    global _PROGRAM
    if _PROGRAM is None:
        _PROGRAM = build_program(CFG)
    return _PROGRAM


def kernel(x: np.ndarray) -> np.ndarray:
    from concourse.bass_utils import run_bass_kernel_spmd

    c = CFG
    assert x.shape == (c.B, c.C, c.H, c.W), x.shape
    nc = _program()
    in_maps = shard_inputs(c, np.asarray(x))
    res = run_bass_kernel_spmd(nc, in_maps, core_ids=list(range(c.ncores)))
    return assemble_out(c, [res.results[k]["out"] for k in range(c.ncores)])
